# revision 26
# baseline (speedup 1.0000x reference)
"""Data-parallel BNN forward kernel for Trainium2 (8 NeuronCores).

Computes (matching the jax reference):
    h  = x @ sign(W1).T + b1          # [B, 100]
    hn = batchnorm(h; batch stats, eps=1e-4) * gamma + beta
    a  = sign(hn)                     # {-1, +1}
    o  = a @ sign(W2).T + b2          # [B, 1000]
    out = log_softmax(o, axis=-1)

Sharding: batch-parallel across 8 cores (4096 rows each), weights
replicated, BN batch statistics combined with one 800-byte AllReduce.

GEMM1 runs in f16: x is cast f32->f16 on load (DVE/ACT copies for 3 of
each 4 batch sub-rows, a gpsimd casting DMA for the 4th), transposed
on the PE in f16 (1.0 c/r vs 1.5 for f32r, measured 117ns per 128x128
tile), and contracted with sign(W1) f16 at ~0.5 cycles/row for N=512
moving tiles.  End-to-end rel err with f16 x is ~5.6e-3 (CPU-simulated
and HW-verified), inside the 2e-2 gate.  x loads use 8KB per-partition
descriptors (~400 GB/s pool rate vs ~330 at 4KB).

Weights are prepared (sign + transpose + bias-row packing) ONCE -- they
are constant across reps -- instead of per rep.

The per-rep structure is software-pipelined: the BN AllReduce of rep i
and the entire sign/GEMM2/log_softmax/output tail of rep i execute
interleaved with GEMM1 of rep i+1 (one tail slice per GEMM1 batch
tile), so neither the ~25us collective latency nor the tail phase
appears on the critical path in steady state.

log_softmax skips the max subtraction: a constant bias -60 keeps exp
args in [-112, -4] (row maxes are 24..56 on this distribution), far
from fp32 trouble.  Per tile: exp+accum on ACT reading PSUM directly,
ln on ACT, fused (o - lse) tensor_scalar on DVE.
"""
import numpy as np

B, D, H, O = 32768, 4096, 100, 1000
NCORES = 8
BC = B // NCORES          # batch rows per core
BN_EPS = 1e-4

TB = 512                  # batch tile (4 x 128 sub-rows)
NBT = BC // TB            # 8 batch tiles per core
DH = 2048                 # feature half per x load
KA = H + 2                # GEMM2 contraction with 2 bias rows
EXPB = 60.0

_CACHE = {}


def _build_nc(reps=1, variant="full"):
    from concourse import bacc, mybir
    import concourse.tile as tile
    from concourse.masks import make_identity

    f32, f16 = mybir.dt.float32, mybir.dt.float16
    AF = mybir.ActivationFunctionType
    ALU = mybir.AluOpType

    class _Bacc(bacc.Bacc):
        """Bacc whose activation-table pass keeps ONE resident func set.

        Every activation this kernel uses (copy/identity/sign/exp/ln)
        lives in the single act_info.json set
        'natural_log_exp_and_others'; remap all loads to that set and
        drop redundant ones so ACT never stalls on a table switch.
        """

        def insert_act_table_loads(self):
            super().insert_act_table_loads()
            from concourse.hw_specs import get_activation_tables
            tables = get_activation_tables(self.m.arch)
            names = list(tables.keys())
            target = names.index("natural_log_exp_and_others")
            allowed = tables["natural_log_exp_and_others"]
            used = {
                i.func
                for b in self.main_func.blocks
                for i in b.instructions
                if isinstance(i, mybir.InstActivation)
            }
            if not used.issubset(allowed):
                return  # fall back to stock behaviour
            for blk in self.main_func.blocks:
                kept = []
                seen = False
                for ins in blk.instructions:
                    if isinstance(ins, mybir.InstLoadActFuncSet):
                        si = ins.sync_info
                        if si is not None and (len(si.on_wait) > 0
                                               or len(si.on_update) > 0):
                            kept.append(ins)  # never drop synced insts
                            continue
                        if seen:
                            continue
                        ins.act_func_set_id = target
                        kept.append(ins)
                        seen = True
                    else:
                        kept.append(ins)
                blk.instructions = kept

    nc = _Bacc(num_devices=NCORES)

    x = nc.dram_tensor("x", [BC, D], f32, kind="ExternalInput")
    W1 = nc.dram_tensor("W1", [H, D], f32, kind="ExternalInput")
    b1 = nc.dram_tensor("b1", [H], f32, kind="ExternalInput")
    gamma = nc.dram_tensor("gamma", [H], f32, kind="ExternalInput")
    beta = nc.dram_tensor("beta", [H], f32, kind="ExternalInput")
    W2 = nc.dram_tensor("W2", [O, H], f32, kind="ExternalInput")
    b2 = nc.dram_tensor("b2", [O], f32, kind="ExternalInput")
    out = nc.dram_tensor("out", [BC, O], f32, kind="ExternalOutput")

    cc_in = nc.dram_tensor("cc_in", [H, 2], f32)
    cc_out = nc.dram_tensor("cc_out", [H, 2], f32, addr_space="Shared")

    NKC = D // 128            # 32 feature chunks
    NCH = DH // 128           # 16 chunks per half

    with tile.TileContext(nc) as tc:
        with (
            tc.tile_pool(name="const", bufs=1) as cp,
            tc.tile_pool(name="xload", bufs=3) as xp,
            tc.tile_pool(name="work", bufs=2) as hp,
            tc.tile_pool(name="softmax", bufs=2) as sp,
            tc.tile_pool(name="ps", bufs=2, space="PSUM") as ps,
        ):
            # ---------------- one-time prep (weights are rep-constant) --
            ident16 = cp.tile([128, 128], f16)
            make_identity(nc, ident16)

            b1_t = cp.tile([H, 1], f32)
            nc.sync.dma_start(out=b1_t, in_=b1[:].unsqueeze(1))
            gamma_t = cp.tile([H, 1], f32)
            nc.sync.dma_start(out=gamma_t, in_=gamma[:].unsqueeze(1))
            beta_t = cp.tile([H, 1], f32)
            nc.sync.dma_start(out=beta_t, in_=beta[:].unsqueeze(1))
            eps_t = cp.tile([H, 1], f32)
            nc.vector.memset(eps_t, BN_EPS)
            expb_t = cp.tile([128, 1], f32)
            nc.vector.memset(expb_t, -EXPB)

            # sign(W1) transposed chunks, f16: sw1t[:, kc, j] = sW1[j, kc*128+:]
            sw1n = cp.tile([H, D], f16)
            for wc in range(4):
                w1_sb = xp.tile([H, 1024], f32, tag="w1l", bufs=1)
                nc.sync.dma_start(out=w1_sb,
                                  in_=W1[:, wc * 1024:(wc + 1) * 1024])
                nc.scalar.activation(out=sw1n[:, wc * 1024:(wc + 1) * 1024],
                                     in_=w1_sb, func=AF.Sign)
            sw1t = cp.tile([128, NKC, H], f16)
            for kc in range(NKC):
                pt = ps.tile([128, H], f16, tag="xt", bufs=3)
                nc.tensor.transpose(
                    pt, sw1n[:, kc * 128:(kc + 1) * 128], ident16[:H, :H])
                nc.scalar.copy(out=sw1t[:, kc, :], in_=pt)

            # sign(W2).T with two bias rows, padded to 1024 columns
            sw2aug = cp.tile([KA, 1024], f16)
            nc.vector.memset(sw2aug[0:H, O:1024], 0.0)
            for i in range(8):
                wt = xp.tile([125, H], f32, tag="w2l", bufs=2)
                nc.sync.dma_start(out=wt, in_=W2[i * 125:(i + 1) * 125, :])
                wsg = xp.tile([125, H], f16, tag="w2s", bufs=2)
                nc.scalar.activation(out=wsg, in_=wt, func=AF.Sign)
                pt = ps.tile([H, 125], f16, tag="xt", bufs=3)
                nc.tensor.transpose(pt, wsg, ident16[:125, :125])
                nc.vector.tensor_copy(
                    out=sw2aug[0:H, i * 125:(i + 1) * 125], in_=pt)
            b2_sb = cp.tile([1, O], f32)
            nc.sync.dma_start(out=b2_sb, in_=b2[:].unsqueeze(0))
            b2hi = cp.tile([1, 1024], f16)
            nc.vector.memset(b2hi, -200.0)
            nc.scalar.copy(out=b2hi[:, 0:O], in_=b2_sb)
            b2lo = cp.tile([1, 1024], f16)
            nc.vector.memset(b2lo, 0.0)
            nc.vector.tensor_tensor(
                out=b2lo[:, 0:O], in0=b2_sb, in1=b2hi[:, 0:O],
                op=ALU.subtract)
            nc.sync.dma_start(out=sw2aug[H:H + 1, :], in_=b2hi)
            nc.sync.dma_start(out=sw2aug[H + 1:H + 2, :], in_=b2lo)

            # ---------------- per-rep tail (sign/GEMM2/softmax/out) -----
            # Emitted one slice per GEMM1 batch tile of the NEXT rep so
            # it overlaps; the closure owns all cross-slice state.
            def make_tail(hT):
                # slice s (2..9) processes t4 = s-2; the AllReduce gets
                # ~2 batch tiles (~60us) of slack before slice 2 needs
                # its result.  Output DMAs are queued and flushed one
                # slice LATE on the sync/scalar queues so a slow softmax
                # chain can never head-of-line-block the x feed.
                st = {"slag": None, "atiles": {}, "Asc": None, "Bv": None,
                      "outq": [], "nout": 0}

                def emit_sign(t4):
                    aT4 = sp.tile([KA, TB], f16, tag="aT4", bufs=3)
                    nc.vector.memset(aT4[96:KA, :], 1.0)
                    nc.scalar.activation(
                        out=aT4[0:H, :],
                        in_=hT[:, t4 * TB:(t4 + 1) * TB], func=AF.Sign,
                        scale=st["Asc"], bias=st["Bv"])
                    st["atiles"][t4] = aT4

                def flush_outs(seam=False):
                    for t_, res1_ in st["outq"]:
                        if seam:
                            eng = nc.sync if st["nout"] % 2 == 0 else nc.scalar
                            st["nout"] += 1
                        else:
                            eng = nc.gpsimd
                        eng.dma_start(
                            out=out[t_ * 128:(t_ + 1) * 128, :], in_=res1_)
                    st["outq"] = []

                def emit_lagged():
                    if st["slag"] is None:
                        return
                    t_, o0_, o1_, lse_ = st["slag"]
                    neglse = sp.tile([128, 1], f32, tag="negl", bufs=2)
                    nc.vector.tensor_scalar(out=neglse, in0=lse_,
                                            scalar1=EXPB, scalar2=-1.0,
                                            op0=ALU.add, op1=ALU.mult)
                    res1 = sp.tile([128, O], f32, tag="res1", bufs=6)
                    nc.vector.tensor_scalar(
                        out=res1[:, 0:512], in0=o0_, scalar1=neglse,
                        scalar2=None, op0=ALU.add)
                    nc.vector.tensor_scalar(
                        out=res1[:, 512:O], in0=o1_[:, 0:488],
                        scalar1=neglse, scalar2=None, op0=ALU.add)
                    st["outq"].append((t_, res1))
                    st["slag"] = None

                def slice_fn(s):
                    # s in 0..10; slices 0-2 idle (AllReduce in flight).
                    if s < 3:
                        return
                    flush_outs(seam=(s > NBT - 1))
                    if s == 3:
                        # AllReduce readback + BN affine coefficients
                        g = sp.tile([H, 2], f32, tag="g", bufs=2)
                        nc.gpsimd.dma_start(out=g, in_=cc_out[:, :])
                        mu = sp.tile([H, 1], f32, tag="mu", bufs=2)
                        nc.vector.tensor_copy(out=mu, in_=g[:, 0:1])
                        musq = sp.tile([H, 1], f32, tag="musq", bufs=2)
                        nc.vector.tensor_mul(out=musq, in0=mu, in1=mu)
                        varg = sp.tile([H, 1], f32, tag="varg", bufs=2)
                        nc.vector.tensor_sub(out=varg, in0=g[:, 1:2],
                                             in1=musq)
                        lnv = sp.tile([H, 1], f32, tag="lnv", bufs=2)
                        nc.scalar.activation(out=lnv, in_=varg, func=AF.Ln,
                                             bias=eps_t)
                        rstd = sp.tile([H, 1], f32, tag="rstd", bufs=2)
                        nc.scalar.activation(out=rstd, in_=lnv, func=AF.Exp,
                                             scale=-0.5)
                        Asc = sp.tile([H, 1], f32, tag="Asc", bufs=2)
                        nc.vector.tensor_mul(out=Asc, in0=rstd, in1=gamma_t)
                        muA = sp.tile([H, 1], f32, tag="muA", bufs=2)
                        nc.vector.tensor_mul(out=muA, in0=mu, in1=Asc)
                        Bv = sp.tile([H, 1], f32, tag="Bv", bufs=2)
                        nc.vector.tensor_sub(out=Bv, in0=beta_t, in1=muA)
                        st.update(Asc=Asc, Bv=Bv)
                        emit_sign(0)
                    t4 = s - 3
                    if t4 + 1 < NBT:
                        emit_sign(t4 + 1)
                    aT4 = st["atiles"].pop(t4)
                    for ti in range(4):
                        t = t4 * 4 + ti
                        o2 = ps.tile([128, 2, 512], f32, tag="o", bufs=2)
                        asl = aT4[:, ti * 128:(ti + 1) * 128]
                        nc.tensor.matmul(o2[:, 0, :], asl, sw2aug[:, 0:512],
                                         start=True, stop=True)
                        nc.tensor.matmul(o2[:, 1, :], asl,
                                         sw2aug[:, 512:1024],
                                         start=True, stop=True)
                        e0 = sp.tile([128, 2, 512], f32, tag="e0", bufs=1)
                        s0 = sp.tile([128, 1], f32, tag="s0", bufs=2)
                        nc.scalar.activation(out=e0, in_=o2, func=AF.Exp,
                                             bias=expb_t, accum_out=s0)
                        lse = sp.tile([128, 1], f32, tag="lse", bufs=2)
                        nc.scalar.activation(out=lse, in_=s0, func=AF.Ln)
                        emit_lagged()
                        st["slag"] = (t, o2[:, 0, :], o2[:, 1, :], lse)
                    if t4 == NBT - 1:
                        emit_lagged()
                        flush_outs(seam=True)

                return slice_fn

            # ---------------- rep loop --------------------------------
            tail = None
            for _rep in range(reps):
                hT = hp.tile([H, BC], f32, tag="hT", bufs=2)
                stats = hp.tile([H, NBT, 6], f32, tag="stats", bufs=2)
                for bt in range(NBT):
                    h_ps = ps.tile([H, TB], f32, tag="h", bufs=1)
                    for half in range(2):
                        dsl = slice(half * DH, (half + 1) * DH)
                        # s=0..2 f32 rows via HWDGE (one DMA per 128-row
                        # group for fine-grained pipelining); s=3 is a
                        # gpsimd casting DMA into its OWN tile so the
                        # write never orders against the DVE casts.
                        x16b = xp.tile([128, DH], f16, tag="x16b", bufs=3)
                        nc.gpsimd.dma_start(
                            out=x16b,
                            in_=x[bt * TB + 384:(bt + 1) * TB, dsl])
                        x16 = xp.tile([128, 3, DH], f16, tag="x16", bufs=2)
                        eng = nc.sync if half == 0 else nc.scalar
                        for s in range(3):
                            xb = xp.tile([128, DH], f32, tag="xb", bufs=8,
                                         name=f"xb{s}")
                            eng.dma_start(
                                out=xb,
                                in_=x[bt * TB + s * 128:
                                      bt * TB + (s + 1) * 128, dsl])
                            nc.vector.tensor_copy(out=x16[:, s, :], in_=xb)
                        # chunk pairs: 8 transposes -> 1 evac -> 2 matmuls
                        lags = []

                        def emit_mm(force=False, h_ps=h_ps):
                            while lags and (force or len(lags) > 2):
                                ck_, xr_ = lags.pop(0)
                                nc.tensor.matmul(
                                    h_ps, sw1t[:, ck_, :], xr_,
                                    start=(ck_ == 0), stop=(ck_ == NKC - 1))

                        for q2 in range(NCH // 2):
                            xt_ps = ps.tile([128, 8, 128], f16, tag="xt",
                                            bufs=3)
                            for qq in range(2):
                                c = q2 * 2 + qq
                                for s in range(4):
                                    src = (x16[:, s, c * 128:(c + 1) * 128]
                                           if s < 3 else
                                           x16b[:, c * 128:(c + 1) * 128])
                                    nc.tensor.transpose(
                                        xt_ps[:, qq * 4 + s, :],
                                        src, ident16)
                            xr = hp.tile([128, 2, 512], f16, tag="xr",
                                         bufs=4)
                            if q2 % 2 == 0:
                                nc.scalar.copy(out=xr, in_=xt_ps)
                            else:
                                nc.vector.tensor_copy(out=xr, in_=xt_ps)
                            ckb = half * NCH + q2 * 2
                            lags.append((ckb, xr[:, 0, :]))
                            lags.append((ckb + 1, xr[:, 1, :]))
                            emit_mm()
                        emit_mm(force=True)
                    nc.scalar.activation(
                        out=hT[:, bt * TB:(bt + 1) * TB], in_=h_ps,
                        func=AF.Identity, bias=b1_t)
                    nc.vector.bn_stats(
                        out=stats[:, bt, :],
                        in_=hT[:, bt * TB:(bt + 1) * TB])
                    if tail is not None:
                        tail(bt)

                if tail is not None:
                    for sl in range(NBT, NBT + 3):
                        tail(sl)   # t4 = 5..7 of the previous tail

                # BN aggregate + AllReduce launch (result consumed by the
                # tail during the next rep)
                mv = sp.tile([H, 2], f32, tag="mv", bufs=2)
                nc.vector.bn_aggr(out=mv, in_=stats)
                msq = sp.tile([H, 1], f32, tag="msq", bufs=2)
                nc.vector.tensor_mul(out=msq, in0=mv[:, 0:1], in1=mv[:, 0:1])
                e2 = sp.tile([H, 1], f32, tag="e2", bufs=2)
                nc.vector.tensor_add(out=e2, in0=mv[:, 1:2], in1=msq)
                ccs = sp.tile([H, 2], f32, tag="ccs", bufs=2)
                nc.scalar.mul(out=ccs[:, 0:1], in_=mv[:, 0:1],
                              mul=1.0 / NCORES)
                nc.scalar.mul(out=ccs[:, 1:2], in_=e2, mul=1.0 / NCORES)
                nc.gpsimd.dma_start(out=cc_in[:, :], in_=ccs)
                nc.gpsimd.collective_compute(
                    "AllReduce", ALU.add,
                    replica_groups=[list(range(NCORES))],
                    ins=[cc_in[:, :]], outs=[cc_out[:, :]])
                tail = make_tail(hT)

            # drain the last rep's tail
            for s in range(NBT + 3):
                tail(s)

    nc.finalize()
    return nc


def _get_runner(reps=1, variant="full"):
    """Compile (once) and return a callable running the SPMD kernel."""
    key = ("runner", reps, variant)
    if key in _CACHE:
        return _CACHE[key]

    import jax
    from jax.sharding import Mesh, PartitionSpec
    from concourse import mybir
    from concourse import bass2jax
    from concourse.bass2jax import _bass_exec_p, install_neuronx_cc_hook

    try:
        from jax.shard_map import shard_map
    except Exception:
        from jax.experimental.shard_map import shard_map

    install_neuronx_cc_hook()
    nc = _build_nc(reps=reps, variant=variant)

    partition_name = (nc.partition_id_tensor.name
                      if nc.partition_id_tensor else None)
    in_names, out_names, out_avals = [], [], []
    for alloc in nc.m.functions[0].allocations:
        if not isinstance(alloc, mybir.MemoryLocationSet):
            continue
        name = alloc.memorylocations[0].name
        if alloc.kind == "ExternalInput":
            if name != partition_name:
                in_names.append(name)
        elif alloc.kind == "ExternalOutput":
            out_names.append(name)
            shape = tuple(alloc.tensor_shape)
            dtype = mybir.dt.np(alloc.dtype)
            out_avals.append(jax.core.ShapedArray(shape, dtype))
    n_params = len(in_names)
    all_in_names = list(in_names) + list(out_names)
    if partition_name is not None:
        all_in_names.append(partition_name)

    def _body(*args):
        operands = list(args)
        if partition_name is not None:
            operands.append(bass2jax.partition_id_tensor())
        outs = _bass_exec_p.bind(
            *operands,
            out_avals=tuple(out_avals),
            in_names=tuple(all_in_names),
            out_names=tuple(out_names),
            lowering_input_output_aliases=(),
            sim_require_finite=True,
            sim_require_nnan=True,
            nc=nc,
        )
        return tuple(outs)

    devices = jax.devices()[:NCORES]
    mesh = Mesh(np.asarray(devices), ("core",))
    n_outs = len(out_names)
    in_specs = (PartitionSpec("core"),) * (n_params + n_outs)
    out_specs = (PartitionSpec("core"),) * n_outs
    sharded = jax.jit(
        shard_map(_body, mesh=mesh, in_specs=in_specs, out_specs=out_specs,
                  check_rep=False),
        keep_unused=True,
    )
    zeros = [np.zeros((NCORES * a.shape[0], *a.shape[1:]), a.dtype)
             for a in out_avals]
    runner = {
        "sharded": sharded,
        "in_names": in_names,
        "out_names": out_names,
        "zeros": zeros,
        "mesh": mesh,
    }
    _CACHE[key] = runner
    return runner


def _concat_inputs(inputs):
    """Build the global (n_cores*dim0, ...) arrays the shard_map expects."""
    full = {}
    full["x"] = np.ascontiguousarray(inputs["x"], dtype=np.float32)
    for name in ("W1", "b1", "gamma", "beta", "W2", "b2"):
        a = np.ascontiguousarray(inputs[name], dtype=np.float32)
        full[name] = np.concatenate([a] * NCORES, axis=0)
    return full


def run_on_device(inputs, iters=1, reps=1, variant="full"):
    """Run the kernel; returns (full_output, list_of_exec_wall_times_s)."""
    import time
    import jax
    from jax.sharding import NamedSharding, PartitionSpec

    r = _get_runner(reps=reps, variant=variant)
    full = _concat_inputs(inputs)
    shard = NamedSharding(r["mesh"], PartitionSpec("core"))
    dev_args = [jax.device_put(full[n], shard) for n in r["in_names"]]
    dev_zeros = [jax.device_put(z, shard) for z in r["zeros"]]
    # warmup / compile
    outs = r["sharded"](*dev_args, *dev_zeros)
    jax.block_until_ready(outs)
    times = []
    for _ in range(iters):
        t0 = time.perf_counter()
        outs = r["sharded"](*dev_args, *dev_zeros)
        jax.block_until_ready(outs)
        times.append(time.perf_counter() - t0)
    result = np.asarray(outs[r["out_names"].index("out")])
    return result, times


def kernel(**inputs):
    result, _ = run_on_device(inputs, iters=0)
    return result


if __name__ == "__main__":
    rng = np.random.default_rng(0)
    inputs = {
        "x": rng.standard_normal((B, D), dtype=np.float32),
        "W1": (rng.standard_normal((H, D)) * 0.05).astype(np.float32),
        "b1": (rng.standard_normal(H) * 0.05).astype(np.float32),
        "gamma": np.ones(H, np.float32),
        "beta": np.zeros(H, np.float32),
        "W2": (rng.standard_normal((O, H)) * 0.05).astype(np.float32),
        "b2": (rng.standard_normal(O) * 0.05).astype(np.float32),
    }
    out, times = run_on_device(inputs, iters=3)
    print("out", out.shape, out.dtype)
    print("times:", times)


# revision 29
# speedup vs baseline: 1.0381x; 1.0381x over previous
"""Data-parallel BNN forward kernel for Trainium2 (8 NeuronCores).

Computes (matching the jax reference):
    h  = x @ sign(W1).T + b1          # [B, 100]
    hn = batchnorm(h; batch stats, eps=1e-4) * gamma + beta
    a  = sign(hn)                     # {-1, +1}
    o  = a @ sign(W2).T + b2          # [B, 1000]
    out = log_softmax(o, axis=-1)

Sharding: batch-parallel across 8 cores (4096 rows each), weights
replicated, BN batch statistics combined with one 800-byte AllReduce.

GEMM1 runs in f16: x is cast f32->f16 on load (DVE copies for 3 of
each 4 batch sub-rows, a gpsimd casting DMA into a separate tile for
the 4th -- separate so the writers never serialize), transposed on the
PE in f16 (measured 117ns per 128x128 tile vs 160 for f32r), and
contracted with sign(W1) f16 at ~0.5 cycles/row for N=512 moving
tiles.  End-to-end rel err with f16 x is ~5.6e-3 (CPU-simulated and
HW-verified), inside the 2e-2 gate.  x loads are one DMA per 128-row
sub-group with 8KB per-partition descriptors on a deep (bufs=8) ring,
so the HWDGE pool streams continuously at ~300+ GB/s.

Weights are prepared (sign + transpose + bias-row packing) ONCE -- they
are constant across reps -- instead of per rep.

The per-rep structure is software-pipelined: the BN AllReduce of rep i
and the entire sign/GEMM2/log_softmax/output tail of rep i execute
interleaved with GEMM1 of rep i+1 (one tail slice per GEMM1 batch
tile, 2 slices of slack for the collective), so neither the 20-50us
collective latency nor the tail phase appears on the critical path in
steady state.  All tail DMA (output writes, AllReduce readback) stays
on the gpsimd queue, lagged one slice, so a slow softmax chain can
never head-of-line-block the sync/scalar x-load queues.

log_softmax skips the max subtraction: a constant bias -60 keeps exp
args in [-112, -4] (row maxes are 24..56 on this distribution), far
from fp32 trouble.  Per tile: exp+accum on ACT reading PSUM directly,
ln on ACT, fused (o - lse) tensor_scalar on DVE.
"""
import numpy as np

B, D, H, O = 32768, 4096, 100, 1000
NCORES = 8
BC = B // NCORES          # batch rows per core
BN_EPS = 1e-4

TB = 512                  # batch tile (4 x 128 sub-rows)
NBT = BC // TB            # 8 batch tiles per core
DH = 2048                 # feature half per x load
KA = H + 2                # GEMM2 contraction with 2 bias rows
EXPB = 60.0

_CACHE = {}


def _build_nc(reps=1, variant="full"):
    from concourse import bacc, mybir
    import concourse.tile as tile
    from concourse.masks import make_identity

    f32, f16 = mybir.dt.float32, mybir.dt.float16
    AF = mybir.ActivationFunctionType
    ALU = mybir.AluOpType

    class _Bacc(bacc.Bacc):
        """Bacc whose activation-table pass keeps ONE resident func set.

        Every activation this kernel uses (copy/identity/sign/exp/ln)
        lives in the single act_info.json set
        'natural_log_exp_and_others'; remap all loads to that set and
        drop redundant ones so ACT never stalls on a table switch.
        """

        def insert_act_table_loads(self):
            super().insert_act_table_loads()
            from concourse.hw_specs import get_activation_tables
            tables = get_activation_tables(self.m.arch)
            names = list(tables.keys())
            target = names.index("natural_log_exp_and_others")
            allowed = tables["natural_log_exp_and_others"]
            used = {
                i.func
                for b in self.main_func.blocks
                for i in b.instructions
                if isinstance(i, mybir.InstActivation)
            }
            if not used.issubset(allowed):
                return  # fall back to stock behaviour
            for blk in self.main_func.blocks:
                kept = []
                seen = False
                for ins in blk.instructions:
                    if isinstance(ins, mybir.InstLoadActFuncSet):
                        si = ins.sync_info
                        if si is not None and (len(si.on_wait) > 0
                                               or len(si.on_update) > 0):
                            kept.append(ins)  # never drop synced insts
                            continue
                        if seen:
                            continue
                        ins.act_func_set_id = target
                        kept.append(ins)
                        seen = True
                    else:
                        kept.append(ins)
                blk.instructions = kept

    nc = _Bacc(num_devices=NCORES)

    x = nc.dram_tensor("x", [BC, D], f32, kind="ExternalInput")
    W1 = nc.dram_tensor("W1", [H, D], f32, kind="ExternalInput")
    b1 = nc.dram_tensor("b1", [H], f32, kind="ExternalInput")
    gamma = nc.dram_tensor("gamma", [H], f32, kind="ExternalInput")
    beta = nc.dram_tensor("beta", [H], f32, kind="ExternalInput")
    W2 = nc.dram_tensor("W2", [O, H], f32, kind="ExternalInput")
    b2 = nc.dram_tensor("b2", [O], f32, kind="ExternalInput")
    out = nc.dram_tensor("out", [BC, O], f32, kind="ExternalOutput")

    cc_in = nc.dram_tensor("cc_in", [H, 2], f32)
    cc_out = nc.dram_tensor("cc_out", [H, 2], f32, addr_space="Shared")

    NKC = D // 128            # 32 feature chunks
    NCH = DH // 128           # 16 chunks per half

    with tile.TileContext(nc) as tc:
        with (
            tc.tile_pool(name="const", bufs=1) as cp,
            tc.tile_pool(name="xload", bufs=3) as xp,
            tc.tile_pool(name="work", bufs=2) as hp,
            tc.tile_pool(name="softmax", bufs=2) as sp,
            tc.tile_pool(name="ps", bufs=2, space="PSUM") as ps,
        ):
            # ---------------- one-time prep (weights are rep-constant) --
            ident16 = cp.tile([128, 128], f16)
            make_identity(nc, ident16)

            b1_t = cp.tile([H, 1], f32)
            nc.sync.dma_start(out=b1_t, in_=b1[:].unsqueeze(1))
            gamma_t = cp.tile([H, 1], f32)
            nc.sync.dma_start(out=gamma_t, in_=gamma[:].unsqueeze(1))
            beta_t = cp.tile([H, 1], f32)
            nc.sync.dma_start(out=beta_t, in_=beta[:].unsqueeze(1))
            eps_t = cp.tile([H, 1], f32)
            nc.vector.memset(eps_t, BN_EPS)
            expb_t = cp.tile([128, 1], f32)
            nc.vector.memset(expb_t, -EXPB)

            # sign(W1) transposed chunks, f16: sw1t[:, kc, j] = sW1[j, kc*128+:]
            sw1n = cp.tile([H, D], f16)
            for wc in range(4):
                w1_sb = xp.tile([H, 1024], f32, tag="w1l", bufs=1)
                nc.sync.dma_start(out=w1_sb,
                                  in_=W1[:, wc * 1024:(wc + 1) * 1024])
                nc.scalar.activation(out=sw1n[:, wc * 1024:(wc + 1) * 1024],
                                     in_=w1_sb, func=AF.Sign)
            sw1t = cp.tile([128, NKC, H], f16)
            for kc in range(NKC):
                pt = ps.tile([128, H], f16, tag="xt", bufs=3)
                nc.tensor.transpose(
                    pt, sw1n[:, kc * 128:(kc + 1) * 128], ident16[:H, :H])
                nc.scalar.copy(out=sw1t[:, kc, :], in_=pt)

            # sign(W2).T with two bias rows, padded to 1024 columns
            sw2aug = cp.tile([KA, 1024], f16)
            nc.vector.memset(sw2aug[0:H, O:1024], 0.0)
            for i in range(8):
                wt = xp.tile([125, H], f32, tag="w2l", bufs=2)
                nc.sync.dma_start(out=wt, in_=W2[i * 125:(i + 1) * 125, :])
                wsg = xp.tile([125, H], f16, tag="w2s", bufs=2)
                nc.scalar.activation(out=wsg, in_=wt, func=AF.Sign)
                pt = ps.tile([H, 125], f16, tag="xt", bufs=3)
                nc.tensor.transpose(pt, wsg, ident16[:125, :125])
                nc.vector.tensor_copy(
                    out=sw2aug[0:H, i * 125:(i + 1) * 125], in_=pt)
            b2_sb = cp.tile([1, O], f32)
            nc.sync.dma_start(out=b2_sb, in_=b2[:].unsqueeze(0))
            b2hi = cp.tile([1, 1024], f16)
            nc.vector.memset(b2hi, -200.0)
            nc.scalar.copy(out=b2hi[:, 0:O], in_=b2_sb)
            b2lo = cp.tile([1, 1024], f16)
            nc.vector.memset(b2lo, 0.0)
            nc.vector.tensor_tensor(
                out=b2lo[:, 0:O], in0=b2_sb, in1=b2hi[:, 0:O],
                op=ALU.subtract)
            nc.sync.dma_start(out=sw2aug[H:H + 1, :], in_=b2hi)
            nc.sync.dma_start(out=sw2aug[H + 1:H + 2, :], in_=b2lo)

            # ---------------- per-rep tail (sign/GEMM2/softmax/out) -----
            # Emitted one slice per GEMM1 batch tile of the NEXT rep so
            # it overlaps; the closure owns all cross-slice state.
            def make_tail(hT):
                # slice s (2..9) processes t4 = s-2; the AllReduce gets
                # ~2 batch tiles (~60us) of slack before slice 2 needs
                # its result.  Output DMAs are queued and flushed one
                # slice LATE on the sync/scalar queues so a slow softmax
                # chain can never head-of-line-block the x feed.
                st = {"slag": None, "atiles": {}, "Asc": None, "Bv": None,
                      "outq": [], "nout": 0}

                def emit_sign(t4):
                    aT4 = sp.tile([KA, TB], f16, tag="aT4", bufs=3)
                    nc.vector.memset(aT4[96:KA, :], 1.0)
                    nc.scalar.activation(
                        out=aT4[0:H, :],
                        in_=hT[:, t4 * TB:(t4 + 1) * TB], func=AF.Sign,
                        scale=st["Asc"], bias=st["Bv"])
                    st["atiles"][t4] = aT4

                def flush_outs(seam=False):
                    for t_, res1_ in st["outq"]:
                        nc.gpsimd.dma_start(
                            out=out[t_ * 128:(t_ + 1) * 128, :], in_=res1_)
                    st["outq"] = []

                def emit_lagged():
                    if st["slag"] is None:
                        return
                    t_, o0_, o1_, lse_ = st["slag"]
                    neglse = sp.tile([128, 1], f32, tag="negl", bufs=2)
                    nc.vector.tensor_scalar(out=neglse, in0=lse_,
                                            scalar1=EXPB, scalar2=-1.0,
                                            op0=ALU.add, op1=ALU.mult)
                    res1 = sp.tile([128, O], f32, tag="res1", bufs=6)
                    nc.vector.tensor_scalar(
                        out=res1[:, 0:512], in0=o0_, scalar1=neglse,
                        scalar2=None, op0=ALU.add)
                    nc.vector.tensor_scalar(
                        out=res1[:, 512:O], in0=o1_[:, 0:488],
                        scalar1=neglse, scalar2=None, op0=ALU.add)
                    st["outq"].append((t_, res1))
                    st["slag"] = None

                def slice_fn(s):
                    # s in 0..10; slices 0-2 idle (AllReduce in flight).
                    if s < 3:
                        return
                    flush_outs(seam=(s > NBT - 1))
                    if s == 3:
                        # AllReduce readback + BN affine coefficients
                        g = sp.tile([H, 2], f32, tag="g", bufs=2)
                        nc.gpsimd.dma_start(out=g, in_=cc_out[:, :])
                        mu = sp.tile([H, 1], f32, tag="mu", bufs=2)
                        nc.vector.tensor_copy(out=mu, in_=g[:, 0:1])
                        musq = sp.tile([H, 1], f32, tag="musq", bufs=2)
                        nc.vector.tensor_mul(out=musq, in0=mu, in1=mu)
                        varg = sp.tile([H, 1], f32, tag="varg", bufs=2)
                        nc.vector.tensor_sub(out=varg, in0=g[:, 1:2],
                                             in1=musq)
                        lnv = sp.tile([H, 1], f32, tag="lnv", bufs=2)
                        nc.scalar.activation(out=lnv, in_=varg, func=AF.Ln,
                                             bias=eps_t)
                        rstd = sp.tile([H, 1], f32, tag="rstd", bufs=2)
                        nc.scalar.activation(out=rstd, in_=lnv, func=AF.Exp,
                                             scale=-0.5)
                        Asc = sp.tile([H, 1], f32, tag="Asc", bufs=2)
                        nc.vector.tensor_mul(out=Asc, in0=rstd, in1=gamma_t)
                        muA = sp.tile([H, 1], f32, tag="muA", bufs=2)
                        nc.vector.tensor_mul(out=muA, in0=mu, in1=Asc)
                        Bv = sp.tile([H, 1], f32, tag="Bv", bufs=2)
                        nc.vector.tensor_sub(out=Bv, in0=beta_t, in1=muA)
                        st.update(Asc=Asc, Bv=Bv)
                        emit_sign(0)
                    t4 = s - 3
                    if t4 + 1 < NBT:
                        emit_sign(t4 + 1)
                    aT4 = st["atiles"].pop(t4)
                    for ti in range(4):
                        t = t4 * 4 + ti
                        o2 = ps.tile([128, 2, 512], f32, tag="o", bufs=2)
                        asl = aT4[:, ti * 128:(ti + 1) * 128]
                        nc.tensor.matmul(o2[:, 0, :], asl, sw2aug[:, 0:512],
                                         start=True, stop=True)
                        nc.tensor.matmul(o2[:, 1, :], asl,
                                         sw2aug[:, 512:1024],
                                         start=True, stop=True)
                        e0 = sp.tile([128, 2, 512], f32, tag="e0", bufs=1)
                        s0 = sp.tile([128, 1], f32, tag="s0", bufs=2)
                        nc.scalar.activation(out=e0, in_=o2, func=AF.Exp,
                                             bias=expb_t, accum_out=s0)
                        lse = sp.tile([128, 1], f32, tag="lse", bufs=2)
                        nc.scalar.activation(out=lse, in_=s0, func=AF.Ln)
                        emit_lagged()
                        st["slag"] = (t, o2[:, 0, :], o2[:, 1, :], lse)
                    if t4 == NBT - 1:
                        emit_lagged()
                        flush_outs(seam=True)

                return slice_fn

            # ---------------- rep loop --------------------------------
            tail = None
            for _rep in range(reps):
                hT = hp.tile([H, BC], f32, tag="hT", bufs=2)
                stats = hp.tile([H, NBT, 6], f32, tag="stats", bufs=2)
                for bt in range(NBT):
                    h_ps = ps.tile([H, TB], f32, tag="h", bufs=1)
                    for half in range(2):
                        dsl = slice(half * DH, (half + 1) * DH)
                        # s=0..2 f32 rows via HWDGE (one DMA per 128-row
                        # group for fine-grained pipelining); s=3 is a
                        # gpsimd casting DMA into its OWN tile so the
                        # write never orders against the DVE casts.
                        x16b = xp.tile([128, DH], f16, tag="x16b", bufs=3)
                        nc.gpsimd.dma_start(
                            out=x16b,
                            in_=x[bt * TB + 384:(bt + 1) * TB, dsl])
                        x16 = xp.tile([128, 3, DH], f16, tag="x16", bufs=2)
                        eng = nc.sync if half == 0 else nc.scalar
                        for s in range(3):
                            xb = xp.tile([128, DH], f32, tag="xb", bufs=8,
                                         name=f"xb{s}")
                            eng.dma_start(
                                out=xb,
                                in_=x[bt * TB + s * 128:
                                      bt * TB + (s + 1) * 128, dsl])
                            nc.vector.tensor_copy(out=x16[:, s, :], in_=xb)
                        # chunk pairs: 8 transposes -> 1 evac -> 2 matmuls
                        lags = []

                        def emit_mm(force=False, h_ps=h_ps):
                            while lags and (force or len(lags) > 2):
                                ck_, xr_ = lags.pop(0)
                                nc.tensor.matmul(
                                    h_ps, sw1t[:, ck_, :], xr_,
                                    start=(ck_ == 0), stop=(ck_ == NKC - 1))

                        for q2 in range(NCH // 2):
                            xt_ps = ps.tile([128, 8, 128], f16, tag="xt",
                                            bufs=3)
                            for qq in range(2):
                                c = q2 * 2 + qq
                                for s in range(4):
                                    src = (x16[:, s, c * 128:(c + 1) * 128]
                                           if s < 3 else
                                           x16b[:, c * 128:(c + 1) * 128])
                                    nc.tensor.transpose(
                                        xt_ps[:, qq * 4 + s, :],
                                        src, ident16)
                            xr = hp.tile([128, 2, 512], f16, tag="xr",
                                         bufs=4)
                            if q2 % 2 == 0:
                                nc.scalar.copy(out=xr, in_=xt_ps)
                            else:
                                nc.vector.tensor_copy(out=xr, in_=xt_ps)
                            ckb = half * NCH + q2 * 2
                            lags.append((ckb, xr[:, 0, :]))
                            lags.append((ckb + 1, xr[:, 1, :]))
                            emit_mm()
                        emit_mm(force=True)
                    nc.scalar.activation(
                        out=hT[:, bt * TB:(bt + 1) * TB], in_=h_ps,
                        func=AF.Identity, bias=b1_t)
                    nc.vector.bn_stats(
                        out=stats[:, bt, :],
                        in_=hT[:, bt * TB:(bt + 1) * TB])
                    if tail is not None:
                        tail(bt)

                if tail is not None:
                    for sl in range(NBT, NBT + 3):
                        tail(sl)   # t4 = 5..7 of the previous tail

                # BN aggregate + AllReduce launch (result consumed by the
                # tail during the next rep)
                mv = sp.tile([H, 2], f32, tag="mv", bufs=2)
                nc.vector.bn_aggr(out=mv, in_=stats)
                msq = sp.tile([H, 1], f32, tag="msq", bufs=2)
                nc.vector.tensor_mul(out=msq, in0=mv[:, 0:1], in1=mv[:, 0:1])
                e2 = sp.tile([H, 1], f32, tag="e2", bufs=2)
                nc.vector.tensor_add(out=e2, in0=mv[:, 1:2], in1=msq)
                ccs = sp.tile([H, 2], f32, tag="ccs", bufs=2)
                nc.scalar.mul(out=ccs[:, 0:1], in_=mv[:, 0:1],
                              mul=1.0 / NCORES)
                nc.scalar.mul(out=ccs[:, 1:2], in_=e2, mul=1.0 / NCORES)
                nc.gpsimd.dma_start(out=cc_in[:, :], in_=ccs)
                nc.gpsimd.collective_compute(
                    "AllReduce", ALU.add,
                    replica_groups=[list(range(NCORES))],
                    ins=[cc_in[:, :]], outs=[cc_out[:, :]])
                tail = make_tail(hT)

            # drain the last rep's tail
            for s in range(NBT + 3):
                tail(s)

    nc.finalize()
    return nc


def _get_runner(reps=1, variant="full"):
    """Compile (once) and return a callable running the SPMD kernel."""
    key = ("runner", reps, variant)
    if key in _CACHE:
        return _CACHE[key]

    import jax
    from jax.sharding import Mesh, PartitionSpec
    from concourse import mybir
    from concourse import bass2jax
    from concourse.bass2jax import _bass_exec_p, install_neuronx_cc_hook

    try:
        from jax.shard_map import shard_map
    except Exception:
        from jax.experimental.shard_map import shard_map

    install_neuronx_cc_hook()
    nc = _build_nc(reps=reps, variant=variant)

    partition_name = (nc.partition_id_tensor.name
                      if nc.partition_id_tensor else None)
    in_names, out_names, out_avals = [], [], []
    for alloc in nc.m.functions[0].allocations:
        if not isinstance(alloc, mybir.MemoryLocationSet):
            continue
        name = alloc.memorylocations[0].name
        if alloc.kind == "ExternalInput":
            if name != partition_name:
                in_names.append(name)
        elif alloc.kind == "ExternalOutput":
            out_names.append(name)
            shape = tuple(alloc.tensor_shape)
            dtype = mybir.dt.np(alloc.dtype)
            out_avals.append(jax.core.ShapedArray(shape, dtype))
    n_params = len(in_names)
    all_in_names = list(in_names) + list(out_names)
    if partition_name is not None:
        all_in_names.append(partition_name)

    def _body(*args):
        operands = list(args)
        if partition_name is not None:
            operands.append(bass2jax.partition_id_tensor())
        outs = _bass_exec_p.bind(
            *operands,
            out_avals=tuple(out_avals),
            in_names=tuple(all_in_names),
            out_names=tuple(out_names),
            lowering_input_output_aliases=(),
            sim_require_finite=True,
            sim_require_nnan=True,
            nc=nc,
        )
        return tuple(outs)

    devices = jax.devices()[:NCORES]
    mesh = Mesh(np.asarray(devices), ("core",))
    n_outs = len(out_names)
    in_specs = (PartitionSpec("core"),) * (n_params + n_outs)
    out_specs = (PartitionSpec("core"),) * n_outs
    sharded = jax.jit(
        shard_map(_body, mesh=mesh, in_specs=in_specs, out_specs=out_specs,
                  check_rep=False),
        keep_unused=True,
    )
    zeros = [np.zeros((NCORES * a.shape[0], *a.shape[1:]), a.dtype)
             for a in out_avals]
    runner = {
        "sharded": sharded,
        "in_names": in_names,
        "out_names": out_names,
        "zeros": zeros,
        "mesh": mesh,
    }
    _CACHE[key] = runner
    return runner


def _concat_inputs(inputs):
    """Build the global (n_cores*dim0, ...) arrays the shard_map expects."""
    full = {}
    full["x"] = np.ascontiguousarray(inputs["x"], dtype=np.float32)
    for name in ("W1", "b1", "gamma", "beta", "W2", "b2"):
        a = np.ascontiguousarray(inputs[name], dtype=np.float32)
        full[name] = np.concatenate([a] * NCORES, axis=0)
    return full


def run_on_device(inputs, iters=1, reps=1, variant="full"):
    """Run the kernel; returns (full_output, list_of_exec_wall_times_s)."""
    import time
    import jax
    from jax.sharding import NamedSharding, PartitionSpec

    r = _get_runner(reps=reps, variant=variant)
    full = _concat_inputs(inputs)
    shard = NamedSharding(r["mesh"], PartitionSpec("core"))
    dev_args = [jax.device_put(full[n], shard) for n in r["in_names"]]
    dev_zeros = [jax.device_put(z, shard) for z in r["zeros"]]
    # warmup / compile
    outs = r["sharded"](*dev_args, *dev_zeros)
    jax.block_until_ready(outs)
    times = []
    for _ in range(iters):
        t0 = time.perf_counter()
        outs = r["sharded"](*dev_args, *dev_zeros)
        jax.block_until_ready(outs)
        times.append(time.perf_counter() - t0)
    result = np.asarray(outs[r["out_names"].index("out")])
    return result, times


def kernel(**inputs):
    result, _ = run_on_device(inputs, iters=0)
    return result


if __name__ == "__main__":
    rng = np.random.default_rng(0)
    inputs = {
        "x": rng.standard_normal((B, D), dtype=np.float32),
        "W1": (rng.standard_normal((H, D)) * 0.05).astype(np.float32),
        "b1": (rng.standard_normal(H) * 0.05).astype(np.float32),
        "gamma": np.ones(H, np.float32),
        "beta": np.zeros(H, np.float32),
        "W2": (rng.standard_normal((O, H)) * 0.05).astype(np.float32),
        "b2": (rng.standard_normal(O) * 0.05).astype(np.float32),
    }
    out, times = run_on_device(inputs, iters=3)
    print("out", out.shape, out.dtype)
    print("times:", times)
